# revision 7
# baseline (speedup 1.0000x reference)
"""Cross-attention Bass kernel for Trainium2.

Problem (per batch, data-parallel over 8 batches -> 8 NeuronCores):
    q = query @ W_q          [2048, 64]
    k = key   @ W_k          [2048, 64]
    v = key   @ W_v          [2048, 64]
    scores = q @ k.T         [2048, 2048]
    attn = softmax(scores, axis=-1)
    out = attn @ v           [2048, 64]

Strategy (per core):
  - scores = query @ (W_q W_k^T) @ key^T: precompute A = W_q W_k^T [128,128]
    once, R = A^T-contraction with queryT [128, 2048]; then
    scoresT_t [keys, q] = keyT_t^T @ R. No separate q/k projections.
  - queryT / keyT [D, L] loaded straight from DRAM with transposed
    (strided) DMA access patterns - no PE transposes or PSUM round-trips.
  - float32r everywhere on the PE (1 cycle/row at N=512 vs 4 for fp32).
  - softmax exp split across engines per tile: ACT does cols 0:768 from
    PSUM, DVE copies cols 768:1024 to SBUF, Pool (gpsimd) computes
    e^x via tensor_tensor(pow) on them.
  - v_aug [128, 65] per key tile ([v | ones]); the ones column makes the
    attn@v matmul accumulate the softmax denominator in row 64.
  - attn@v accumulates outT [65, chunk] in PSUM over the 16 key tiles,
    software-pipelined two steps behind the score matmuls.
  - epilogue: PE-transpose [65, 128] slices back, reciprocal + per-row
    scale on DVE, DMA out on the Pool ring.
  - junk warm-up matmuls burn the PE ramp (1.2 -> 2.4 GHz) during the
    input-DMA window.
"""

import numpy as np

import concourse.bass as bass
import concourse.bacc as bacc
import concourse.mybir as mybir
import concourse.tile as tile
from concourse import bass_utils
from concourse.masks import make_identity

F32 = mybir.dt.float32
F32R = mybir.dt.float32r
AF = mybir.ActivationFunctionType
ALU = mybir.AluOpType

B = 8
L = 2048
D = 128
E = 64
NT = L // 128          # 16 key tiles
CHUNK = 1024           # query chunk (PSUM budget)
NCHUNK = L // CHUNK    # 2
ACT_COLS = 768         # exp columns done on ACT; rest go DVE-copy + Pool pow
LAG = 2                # attn@v pipeline lag behind scores


def _build(nc: bass.Bass, tc: tile.TileContext, out, query, key, wq, wk, wv, ctx):
    const = ctx.enter_context(tc.tile_pool(name="const", bufs=1))
    ident = const.tile([128, 128], F32)
    make_identity(nc, ident[:])

    # Warm the ACT function-table early (PSEUDO_LOAD_ACT_FUNC_SET ~2.7us).
    warm = const.tile([128, 1], F32)
    nc.vector.memset(warm[:], 0.0)
    nc.scalar.activation(warm[:], warm[:], AF.Exp)

    junk32 = const.tile([128, 512], F32)
    nc.vector.memset(junk32[:], 0.001)
    junk = const.tile([128, 512], F32R)
    nc.vector.tensor_copy(junk[:], junk32[:])

    econst = const.tile([128, CHUNK - ACT_COLS], F32)
    nc.gpsimd.memset(econst[:], float(np.e))

    vag = const.tile([128, 65 * NT], F32R)   # per-tile [v | ones]
    ones32 = const.tile([128, NT], F32)
    nc.gpsimd.memset(ones32[:], 1.0)
    vagv = vag.rearrange("p (t e) -> p t e", e=65)
    nc.vector.tensor_copy(vagv[:, :, 64:65].squeeze(2), ones32[:])

    # ---------------- input DMA ----------------
    # weights on the Pool ring (cheap issue), kT on SP, qT split ACT/DVE.
    wqT = const.tile([64, 128], F32R)
    wkT = const.tile([64, 128], F32R)
    wvn = const.tile([128, E], F32R)
    nc.gpsimd.dma_start(wqT[:], wq.rearrange("d e -> e d"))
    nc.gpsimd.dma_start(wkT[:], wk.rearrange("d e -> e d"))
    nc.gpsimd.dma_start(wvn[:], wv)

    qTd = const.tile([128, L], F32R)
    kTd = const.tile([128, L], F32R)
    for j in range(4):
        s = slice(512 * j, 512 * (j + 1))
        nc.scalar.dma_start(qTd[:, s], query[s, :].rearrange("t d -> d t"))
        nc.sync.dma_start(kTd[:, s], key[s, :].rearrange("t d -> d t"))

    # ---------------- PE warm-up + prologue ----------------
    R = const.tile([128, L], F32R)

    with tc.tile_pool(name="pwarm", bufs=1, space="PSUM") as pw_pool, \
         tc.tile_pool(name="pj", bufs=2, space="PSUM") as pj_pool, \
         tc.tile_pool(name="pv", bufs=2, space="PSUM") as pv_pool:
        pwt = pw_pool.tile([128, 512], F32, tag="w")
        for i in range(9):
            nc.tensor.matmul(pwt[:], junk[:, 0:128], junk[:], start=True, stop=True)

        # A = W_q @ W_k^T  [128 d, 128 d']
        psA = pj_pool.tile([128, 128], F32, tag="pa")
        nc.tensor.matmul(psA[:], wqT[:], wkT[:], start=True, stop=True)
        A = const.tile([128, 128], F32R)
        nc.vector.tensor_copy(A[:], psA[:])

        # R[d', q] = sum_d A[d, d'] qT[d, q]
        for j in range(4):
            s = slice(512 * j, 512 * (j + 1))
            psR = pj_pool.tile([128, 512], F32, tag="pr")
            nc.tensor.matmul(psR[:], A[:], qTd[:, s], start=True, stop=True)
            nc.vector.tensor_copy(R[:, s], psR[:])

        # vag tiles: v_t = keyT_t^T @ wv, 4 tiles batched per PSUM tile
        for g in range(4):
            psV = pv_pool.tile([128, 4, 128], F32, tag="pv")
            for i in range(4):
                t = 4 * g + i
                nc.tensor.matmul(psV[:, i, 0:64], kTd[:, 128 * t:128 * (t + 1)],
                                 wvn[:], start=True, stop=True)
            vg = vag[:, 260 * g:260 * (g + 1)].rearrange("p (i e) -> p i e", e=65)
            nc.vector.tensor_copy(vg[:, :, 0:64], psV[:, :, 0:64])

    # ---------------- main loop ----------------
    sc_pool = ctx.enter_context(tc.tile_pool(name="sc", bufs=2, space="PSUM"))
    ou_pool = ctx.enter_context(tc.tile_pool(name="ou", bufs=1, space="PSUM"))
    ep_ps = ctx.enter_context(tc.tile_pool(name="epps", bufs=2, space="PSUM"))
    ex_pool = ctx.enter_context(tc.tile_pool(name="ex", bufs=3))
    epi = ctx.enter_context(tc.tile_pool(name="epi", bufs=2))
    rc_pool = ctx.enter_context(tc.tile_pool(name="rc", bufs=2))

    o16 = out.rearrange("(g t p) e -> g p t e", t=4, p=128)  # [4, 128, 4, 64]

    def epilogue(c, pso):
        outT = epi.tile([65, CHUNK], F32, tag="outT")
        nc.vector.tensor_copy(outT[:], pso[:])
        osb = epi.tile([128, 64 * 8], F32, tag="osb")
        for i in range(8):
            pt = ep_ps.tile([128, 65], F32, tag="ept")
            nc.tensor.transpose(pt[:], outT[:, 128 * i:128 * (i + 1)],
                                ident[0:65, 0:65])
            rec = rc_pool.tile([128, 1], F32, tag="rc")
            nc.vector.reciprocal(rec[:], pt[:, 64:65])
            nc.vector.tensor_scalar_mul(osb[:, 64 * i:64 * (i + 1)],
                                        pt[:, 0:64], rec[:])
        for h in range(2):
            nc.gpsimd.dma_start(
                o16[2 * c + h],
                osb[:, 256 * h:256 * (h + 1)].rearrange("p (t e) -> p t e", e=64))

    for c in range(NCHUNK):
        tiles_ps = [None] * NT
        tiles_ex = [None] * NT
        pso = ou_pool.tile([65, CHUNK], F32, tag="ou")

        def scores_step(t):
            ps = sc_pool.tile([128, CHUNK], F32, tag="sc")
            for j in range(CHUNK // 512):
                qs = slice(CHUNK * c + 512 * j, CHUNK * c + 512 * (j + 1))
                nc.tensor.matmul(ps[:, 512 * j:512 * (j + 1)],
                                 kTd[:, 128 * t:128 * (t + 1)], R[:, qs],
                                 start=True, stop=True)
            ex = ex_pool.tile([128, CHUNK], F32R, tag="ex")
            nc.scalar.activation(ex[:, 0:ACT_COLS], ps[:, 0:ACT_COLS], AF.Exp)
            exs = ex_pool.tile([128, CHUNK - ACT_COLS], F32, tag="exs")
            nc.vector.tensor_copy(exs[:], ps[:, ACT_COLS:CHUNK])
            nc.gpsimd.tensor_tensor(ex[:, ACT_COLS:CHUNK], econst[:], exs[:],
                                    ALU.pow)
            tiles_ps[t] = ps
            tiles_ex[t] = ex

        def attnv_step(t):
            ex = tiles_ex[t]
            for j in range(CHUNK // 512):
                nc.tensor.matmul(pso[:, 512 * j:512 * (j + 1)],
                                 vag[:, 65 * t:65 * t + 65],
                                 ex[:, 512 * j:512 * (j + 1)],
                                 start=(t == 0), stop=(t == NT - 1))

        for t in range(NT + LAG):
            if t < NT:
                scores_step(t)
            if t >= LAG:
                attnv_step(t - LAG)
        epilogue(c, pso)


def build_nc() -> bass.Bass:
    nc = bacc.Bacc("TRN2", target_bir_lowering=False, debug=False,
                   enable_asserts=False, num_devices=B)
    query = nc.dram_tensor("query", [L, D], F32R, kind="ExternalInput").ap()
    key = nc.dram_tensor("key", [L, D], F32R, kind="ExternalInput").ap()
    wq = nc.dram_tensor("W_q", [D, E], F32R, kind="ExternalInput").ap()
    wk = nc.dram_tensor("W_k", [D, E], F32R, kind="ExternalInput").ap()
    wv = nc.dram_tensor("W_v", [D, E], F32R, kind="ExternalInput").ap()
    out = nc.dram_tensor("out", [L, E], F32, kind="ExternalOutput").ap()
    from contextlib import ExitStack
    with tile.TileContext(nc) as tc:
        with ExitStack() as ctx:
            _build(nc, tc, out, query, key, wq, wk, wv, ctx)
    nc.compile()
    return nc


_NC_CACHE = None


def kernel(**inputs) -> np.ndarray:
    global _NC_CACHE
    if _NC_CACHE is None:
        _NC_CACHE = build_nc()
    nc = _NC_CACHE
    q = np.ascontiguousarray(np.asarray(inputs["query"], dtype=np.float32))
    k = np.ascontiguousarray(np.asarray(inputs["key"], dtype=np.float32))
    wq = np.ascontiguousarray(np.asarray(inputs["W_q"], dtype=np.float32))
    wk = np.ascontiguousarray(np.asarray(inputs["W_k"], dtype=np.float32))
    wv = np.ascontiguousarray(np.asarray(inputs["W_v"], dtype=np.float32))
    in_maps = [
        {"query": q[b], "key": k[b], "W_q": wq, "W_k": wk, "W_v": wv}
        for b in range(B)
    ]
    res = bass_utils.run_bass_kernel_spmd(nc, in_maps, core_ids=list(range(B)))
    return np.stack([r["out"] for r in res.results], axis=0)
